# revision 3
# baseline (speedup 1.0000x reference)
"""GRU-D cell (nn_GRUDCell), data-parallel Bass/Tile kernel for 8 TRN2 NeuronCores.

Strategy
--------
Shard the batch dim (16384 -> 8 x 2048) across cores; replicate the 512x512
weights.  Per core, the batch is processed in slices of 512 rows:

 1. SWDGE cast-DMA loads (fp32 HBM -> bf16 SBUF), batch-major [128b, feat].
 2. Element-wise imputation / decay in bf16 on DVE (+ ACT exp), batch-major.
    gamma = exp(-relu(decay) * dt) is computed with the `exp_and_others`
    activation-table set; the gate sigmoids are computed as
    sigmoid(v) = 0.5 + 0.5*tanh(v/2) so that *one* ACT table set (exp+tanh)
    serves the whole kernel (no ~2.7us table reloads).
 3. One DMA-xbar transpose per [128, 512] tile flips x_dec / h_dec into
    feature-major [512f, batch] layout for the matmuls.
 4. TensorE: six 512x512 gate matmuls in bf16, weights stationary (lhsT),
    accumulating X- and H- contributions into the same PSUM bank.
    U_h is pre-scaled by 0.5 on-device so that rh' = (1+tanh(r/2))*h_dec
    (which is 2*r*h_dec) can be fed without an extra scaling pass.
 5. ACT evacuates PSUM through tanh (+ per-partition bias), DVE does the
    final convex combine with fused scalar_tensor_tensor ops, a reverse
    xbar transpose restores batch-major and SWDGE cast-DMA stores fp32.
"""

import numpy as np

import concourse.bacc as bacc
import concourse.mybir as mybir
from concourse.tile import TileContext
from concourse import bass_utils

F = 512               # feature dim == units
P = 128               # partitions
NM = F // P           # 4 feature chunks of 128
N_CORES = 8
B_TOTAL = 16384
BC = B_TOTAL // N_CORES  # 2048 rows per core

FP32 = mybir.dt.float32
BF16 = mybir.dt.bfloat16
AF = mybir.ActivationFunctionType
OP = mybir.AluOpType

_WEIGHT_KEYS = ("W_z", "U_z", "b_z", "W_r", "U_r", "b_r", "W_h", "U_h", "b_h",
                "gamma_x_decay", "gamma_h_decay", "mean_imputation")


def _build(bc=BC, sb=512):
    """Build + compile the per-core kernel for a batch shard of `bc` rows,
    processed in slices of `sb` rows."""
    nslice = bc // sb
    nbt = sb // P

    nc = bacc.Bacc("TRN2", target_bir_lowering=False, debug=False,
                   enable_asserts=False)

    inp = nc.dram_tensor("inputs", [bc, 3 * F], FP32, kind="ExternalInput").ap()
    hpv = nc.dram_tensor("h_prev", [bc, F], FP32, kind="ExternalInput").ap()
    wmats = {
        name: nc.dram_tensor(name, [F, F], FP32, kind="ExternalInput").ap()
        for name in ("W_z", "U_z", "W_r", "U_r", "W_h", "U_h")
    }
    vecs = {
        name: nc.dram_tensor(name, [F], FP32, kind="ExternalInput").ap()
        for name in ("b_z", "b_r", "b_h",
                     "gamma_x_decay", "gamma_h_decay", "mean_imputation")
    }
    out = nc.dram_tensor("out", [bc, F], FP32, kind="ExternalOutput").ap()

    with TileContext(nc) as tc:
        with (
            tc.tile_pool(name="const", bufs=1) as const,
            tc.tile_pool(name="stage", bufs=2) as stage,
            tc.tile_pool(name="raw", bufs=2 * nbt) as rawp,
            tc.tile_pool(name="hp", bufs=2 * nbt) as hpp,
            tc.tile_pool(name="ew", bufs=3) as ewp,
            tc.tile_pool(name="tpose", bufs=2) as tpp,
            tc.tile_pool(name="gates", bufs=2 * NM) as gp,
            tc.tile_pool(name="fin", bufs=3) as fin,
            tc.tile_pool(name="store", bufs=2) as stp,
            tc.tile_pool(name="ps", bufs=8, space="PSUM") as psp,
        ):
            # ---- constants: weights (bf16, K on partitions), U_h scaled 0.5
            w_sb = {}
            for name, ap in wmats.items():
                for k in range(NM):
                    t = const.tile([P, F], BF16, tag=f"w_{name}_{k}")
                    nc.gpsimd.dma_start(t[:], ap[k * P:(k + 1) * P, :])
                    if name == "U_h":
                        nc.vector.tensor_scalar_mul(t[:], t[:], 0.5)
                    w_sb[(name, k)] = t

            # ---- per-feature rows replicated across partitions (bf16)
            def repl_row(name, relu):
                row1 = stage.tile([1, F], FP32, tag="row1")
                nc.gpsimd.dma_start(row1[:], vecs[name].unsqueeze(0))
                rowf = stage.tile([P, F], FP32, tag="rowf")
                nc.gpsimd.partition_broadcast(rowf[:], row1[:])
                rowb = const.tile([P, F], BF16, tag=f"row_{name}")
                if relu:
                    nc.vector.tensor_scalar_max(rowb[:], rowf[:], 0.0)
                else:
                    nc.vector.tensor_copy(rowb[:], rowf[:])
                return rowb

            cx_row = repl_row("gamma_x_decay", relu=True)
            ch_row = repl_row("gamma_h_decay", relu=True)
            mu_row = repl_row("mean_imputation", relu=False)

            # ---- per-chunk biases [128, 1] fp32; z/r biases pre-scaled 0.5
            def bias_chunks(name, half):
                res = []
                for mi in range(NM):
                    t = const.tile([P, 1], FP32, tag=f"b_{name}_{mi}")
                    src = vecs[name][mi * P:(mi + 1) * P].unsqueeze(1)
                    nc.gpsimd.dma_start(t[:], src)
                    if half:
                        nc.vector.tensor_scalar_mul(t[:], t[:], 0.5)
                    res.append(t)
                return res

            bz_h = bias_chunks("b_z", half=True)
            br_h = bias_chunks("b_r", half=True)
            bh_c = bias_chunks("b_h", half=False)

            # ---- main pipeline over batch slices
            for s in range(nslice):
                xdT = tpp.tile([P, NM, sb], BF16, tag="xdT")
                hdT = tpp.tile([P, NM, sb], BF16, tag="hdT")
                for j in range(nbt):
                    r0 = s * sb + j * P
                    raw = rawp.tile([P, 3 * F], BF16, tag="raw")
                    nc.gpsimd.dma_start(raw[:], inp[r0:r0 + P, :])
                    hp = hpp.tile([P, F], BF16, tag="hp")
                    nc.gpsimd.dma_start(hp[:], hpv[r0:r0 + P, :])
                    x, m_, dt = raw[:, :F], raw[:, F:2 * F], raw[:, 2 * F:]

                    u1 = ewp.tile([P, F], BF16, tag="u1")
                    nc.vector.tensor_mul(u1[:], dt, cx_row[:])
                    g1 = ewp.tile([P, F], BF16, tag="g1")
                    nc.scalar.activation(g1[:], u1[:], AF.Exp, scale=-1.0)
                    u2 = ewp.tile([P, F], BF16, tag="u2")
                    nc.vector.tensor_mul(u2[:], dt, ch_row[:])
                    g2 = ewp.tile([P, F], BF16, tag="g2")
                    nc.scalar.activation(g2[:], u2[:], AF.Exp, scale=-1.0)

                    # pq' = (g1 - 1) * (1 - m)   [= -(1-g1)(1-m)]
                    p1 = ewp.tile([P, F], BF16, tag="p1")
                    nc.vector.tensor_scalar(p1[:], m_, -1.0, 1.0, OP.mult, OP.add)
                    pq = ewp.tile([P, F], BF16, tag="pq")
                    nc.vector.scalar_tensor_tensor(pq[:], g1[:], 1.0, p1[:],
                                                   OP.subtract, OP.mult)
                    # x_dec = x + pq' * (x - mu)
                    t_ = ewp.tile([P, F], BF16, tag="t")
                    nc.vector.tensor_sub(t_[:], x, mu_row[:])
                    w_ = ewp.tile([P, F], BF16, tag="w")
                    nc.vector.tensor_mul(w_[:], pq[:], t_[:])
                    xd = ewp.tile([P, F], BF16, tag="xd")
                    nc.vector.tensor_add(xd[:], x, w_[:])
                    # h_dec = g2 * h_prev
                    hd = ewp.tile([P, F], BF16, tag="hd")
                    nc.vector.tensor_mul(hd[:], g2[:], hp[:])

                    nc.sync.dma_start(out=xdT[:, :, j * P:(j + 1) * P], in_=xd[:],
                                      transpose=True)
                    nc.sync.dma_start(out=hdT[:, :, j * P:(j + 1) * P], in_=hd[:],
                                      transpose=True)

                # gates z and r: tanh(0.5*(X@W + H@U) + 0.5*b)
                tzs, trs = [], []
                for wname, uname, bias, outl, tag in (
                        ("W_z", "U_z", bz_h, tzs, "tz"),
                        ("W_r", "U_r", br_h, trs, "tr")):
                    for mi in range(NM):
                        ps = psp.tile([P, sb], FP32, tag="ps")
                        for k in range(NM):
                            nc.tensor.matmul(ps[:], w_sb[(wname, k)][:, mi * P:(mi + 1) * P],
                                             xdT[:, k, :], start=(k == 0), stop=False)
                        for k in range(NM):
                            nc.tensor.matmul(ps[:], w_sb[(uname, k)][:, mi * P:(mi + 1) * P],
                                             hdT[:, k, :], start=False, stop=(k == NM - 1))
                        tg = gp.tile([P, sb], BF16, tag=tag)
                        nc.scalar.activation(tg[:], ps[:], AF.Tanh,
                                             bias=bias[mi][:], scale=0.5)
                        outl.append(tg)

                # rh' = (tr + 1) * h_dec  (= 2 * r * h_dec; U_h pre-scaled 0.5)
                rhs_ = []
                for k in range(NM):
                    rh = gp.tile([P, sb], BF16, tag="rh")
                    nc.vector.scalar_tensor_tensor(rh[:], trs[k][:], 1.0,
                                                   hdT[:, k, :], OP.add, OP.mult)
                    rhs_.append(rh)

                stT = stp.tile([P, nbt, F], BF16, tag="stT")
                for mi in range(NM):
                    ps = psp.tile([P, sb], FP32, tag="ps")
                    for k in range(NM):
                        nc.tensor.matmul(ps[:], w_sb[("W_h", k)][:, mi * P:(mi + 1) * P],
                                         xdT[:, k, :], start=(k == 0), stop=False)
                    for k in range(NM):
                        nc.tensor.matmul(ps[:], w_sb[("U_h", k)][:, mi * P:(mi + 1) * P],
                                         rhs_[k][:], start=False, stop=(k == NM - 1))
                    th = fin.tile([P, sb], BF16, tag="th")
                    nc.scalar.activation(th[:], ps[:], AF.Tanh,
                                         bias=bh_c[mi][:], scale=1.0)
                    # h_new = h_dec + 0.5*(tz+1)*(h_hat - h_dec)
                    d = fin.tile([P, sb], BF16, tag="d")
                    nc.vector.tensor_sub(d[:], th[:], hdT[:, mi, :])
                    q = fin.tile([P, sb], BF16, tag="q")
                    nc.vector.scalar_tensor_tensor(q[:], tzs[mi][:], 1.0, d[:],
                                                   OP.add, OP.mult)
                    hn = fin.tile([P, sb], BF16, tag="hn")
                    nc.vector.scalar_tensor_tensor(hn[:], q[:], 0.5,
                                                   hdT[:, mi, :], OP.mult, OP.add)
                    nc.sync.dma_start(out=stT[:, :, mi * P:(mi + 1) * P], in_=hn[:],
                                      transpose=True)

                for j in range(nbt):
                    r0 = s * sb + j * P
                    nc.gpsimd.dma_start(out[r0:r0 + P, :], stT[:, j, :])

    nc.compile()
    return nc


_NC = None


def _get_nc():
    global _NC
    if _NC is None:
        _NC = _build()
    return _NC


def kernel(**inputs) -> np.ndarray:
    nc = _get_nc()
    inp = np.ascontiguousarray(inputs["inputs"], dtype=np.float32)
    hp = np.ascontiguousarray(inputs["h_prev"], dtype=np.float32)
    b = inp.shape[0]
    inp = inp.reshape(N_CORES, b // N_CORES, 3 * F)
    hp = hp.reshape(N_CORES, b // N_CORES, F)
    shared = {k: np.ascontiguousarray(inputs[k], dtype=np.float32)
              for k in _WEIGHT_KEYS}
    in_maps = [dict(inputs=inp[c], h_prev=hp[c], **shared)
               for c in range(N_CORES)]
    res = bass_utils.run_bass_kernel_spmd(nc, in_maps,
                                          core_ids=list(range(N_CORES)))
    outs = [r["out"] for r in res.results]
    return np.concatenate(outs, axis=0).astype(np.float32)
